# revision 17
# baseline (speedup 1.0000x reference)
"""Trainium2 Bass kernel for nn_ComplexConv2Deffangle4Dxy.

Reference math (per batch b, branch br):
    out[br] = pointwise(w2, depthwise3x3(w1, img[br]))   with zero padding P=1
      br=0 (rot): weights (w1n, w2n) where wn = (wx+wy)^2 / sum((wx+wy)^2)
      br=1 (abs): log-domain: exp(branch(log(img + EPS), w1n, w2n))
      br=2 (x):   weights (w1x, w2x)
      br=3 (y):   weights (w1y, w2y)

Kernel strategy (per NeuronCore, data-parallel over batch B=8 -> 8 cores):
  Fuse depthwise+pointwise into a single 3x3 conv whose weights are the
  outer product  Wf[o, c, k] = w2[o, c] * w1[c, k], computed as
  PSUM-accumulated matmuls over the 9 kernel offsets with
  lhsT = fused weights (K=Cin, M=Cout=128) and rhs = shifted image views.
  Images are zero-padded on the host (pure marshaling) so every shifted
  view is a plain strided AP with no boundary special cases; for the abs
  branch Ln(x*1+EPS) maps the zero padding to log(EPS), exactly matching
  the reference's pad-then-log order.  Weight normalization for the
  rot/abs branches is computed on device (sum via ones-matmul, reciprocal
  on DVE, scale folded into the fused conv weights).

  Scheme "dual": SBUF partitions 0..63 hold the padded image (A), 64..127
  hold the same image shifted down 2 rows (B).  A K=128 matmul at row
  offset r then contracts offset (dh=-1,dw) on the lower half and
  (dh=+1,dw) on the upper half in one instruction.  The dh=0 offsets use
  K=64 matmuls, pairwise packed onto disjoint PE row groups.

  Scheme "hsplit": partitions 0..63 serve output rows 0..31, partitions
  64..127 serve rows 32..63; all matmuls are K=64, issued in pairs on
  disjoint PE row groups (tile_position row tiling).
"""

import sys

for _p in ("/opt/trn_rl_repo",):
    if _p not in sys.path:
        sys.path.insert(0, _p)

import ml_dtypes
import numpy as np

import concourse.bacc as bacc
import concourse.mybir as mybir
import concourse.tile as tile
from concourse import bass_utils

F32 = mybir.dt.float32
F32R = mybir.dt.float32r
BF16 = mybir.dt.bfloat16

EPS = 1e-6
N_CORES = 8
B, NBR, CIN, COUT, H, W = 8, 4, 64, 128, 64, 64
HP, WP = H + 2, W + 2          # host-padded image
HS_ROWS = 35                   # hsplit: padded rows per partition half

# matmul input dtype: "f32r" | "f32" | "bf16"
MM_DTYPE = "bf16"
SCHEME = "hpair"               # "dual" | "hsplit" | "hpair" | "houter" | "hldw"
# output dtype on device: "f32" | "bf16" (bf16 halves the out-DMA traffic;
# host upcasts -- adds <=0.4% rounding, gate is 2e-2)
OUT_DTYPE = "bf16"
# Packing (0,+1) onto the upper PE row group (K=64 at base_partition 64)
# mixed with K=128 matmuls in the same PSUM accumulation group crashes at
# runtime on TRN2 hardware -- keep disabled.
DH0_UPPER_PACK = False
LOOP_ITERS = None              # benchmarking: device-side repeat count
PROBE = ""                     # "" | "no_out" (skip evac+out-DMA) | "no_mm"
TRACE = False
LAST_EXEC_TIME_NS = None
LAST_RESULTS = None

_PROG_CACHE = {}

# walrus's LDWEIGHTS optimization (split weight loads from matmuls so they
# pipeline through the PE reorder window) is hardcoded off in
# bass_utils.bir_verify_and_optimise; expose a switch that rewrites the flag
# inside the compile command.
LDW_OPT = False
_orig_run_command = bass_utils.run_command


def _patched_run_command(cmd, *a, **kw):
    if LDW_OPT and isinstance(cmd, list) and "--enable-ldw-opt=false" in cmd:
        cmd = ["--enable-ldw-opt=true" if c == "--enable-ldw-opt=false" else c for c in cmd]
    return _orig_run_command(cmd, *a, **kw)


bass_utils.run_command = _patched_run_command
if getattr(bass_utils, "bir_verify_and_optimise", None) is not None:
    bass_utils.bir_verify_and_optimise.__globals__["run_command"] = _patched_run_command

BRANCHES = (  # (branch index, weight set, log-domain?, evac engine)
    (2, "x", False, "v"),
    (3, "y", False, "a"),
    (0, "n", False, "v"),
    (1, "n", True, "a"),
)


def _mm_dt():
    return {"f32r": F32R, "f32": F32, "bf16": BF16}[MM_DTYPE]


def _np_in_dt():
    return ml_dtypes.bfloat16 if MM_DTYPE == "bf16" else np.float32


def _out_dt():
    return BF16 if OUT_DTYPE == "bf16" else F32


def _emit(nc, tc, xin_d, w1x_d, w1y_d, w2xT_d, w2yT_d, out_d):
    import contextlib

    mdt = _mm_dt()
    img_rows = HP if SCHEME == "dual" else HS_ROWS
    # houter keeps 8 PSUM accumulators live for a whole branch -> needs all 8
    # banks in one pool; weight-prep reductions then share psp (they finish
    # before the main loop starts).
    psp_bufs = 8 if SCHEME in ("houter", "hldw") else 6
    obp_bufs = 2 if SCHEME in ("hpair", "houter", "hldw") else 6
    with contextlib.ExitStack() as _stack:
        wp = _stack.enter_context(tc.tile_pool(name="wp", bufs=1))
        imgp = _stack.enter_context(tc.tile_pool(name="imgp", bufs=2))
        psp = _stack.enter_context(tc.tile_pool(name="psp", bufs=psp_bufs, space="PSUM"))
        obp = _stack.enter_context(tc.tile_pool(name="obp", bufs=obp_bufs))
        if SCHEME in ("houter", "hldw"):
            psr, red_tag = psp, "ps"
        else:
            psr = _stack.enter_context(tc.tile_pool(name="psr", bufs=2, space="PSUM"))
            red_tag = "red"
        # ---- weight prep -------------------------------------------------
        # All weight/source tiles replicated into both partition halves so
        # per-half fused tiles can be built with partition-local DVE ops.
        w1x_s = wp.tile([2 * CIN, 9], F32, tag="w1x")
        w1y_s = wp.tile([2 * CIN, 9], F32, tag="w1y")
        w2xT_s = wp.tile([2 * CIN, COUT], F32, tag="w2xT")
        w2yT_s = wp.tile([2 * CIN, COUT], F32, tag="w2yT")
        for t, d in (
            (w1x_s, w1x_d),
            (w1y_s, w1y_d),
            (w2xT_s, w2xT_d),
            (w2yT_s, w2yT_d),
        ):
            nc.sync.dma_start(out=t[0:CIN], in_=d)
            nc.sync.dma_start(out=t[CIN : 2 * CIN], in_=d)

        ones_k = wp.tile([CIN, 1], F32, tag="ones_k")
        nc.vector.memset(ones_k[:, :], 1.0)
        ones_m = wp.tile([1, 2 * CIN], F32, tag="ones_m")
        nc.vector.memset(ones_m[:, :], 1.0)
        eps_b = wp.tile([2 * CIN, 1], F32, tag="eps_b")
        nc.vector.memset(eps_b[:, :], float(EPS))
        zero_b = wp.tile([COUT, 1], F32, tag="zero_b")
        nc.vector.memset(zero_b[:, :], 0.0)

        # u1 = (w1x + w1y)^2, u2T = ((w2x + w2y)^2)^T  (both partition halves)
        u1 = wp.tile([2 * CIN, 9], F32, tag="u1")
        nc.vector.tensor_add(u1[:, :], w1x_s[:, :], w1y_s[:, :])
        nc.vector.tensor_mul(u1[:, :], u1[:, :], u1[:, :])
        u2T = wp.tile([2 * CIN, COUT], F32, tag="u2T")
        nc.vector.tensor_add(u2T[:, :], w2xT_s[:, :], w2yT_s[:, :])
        nc.vector.tensor_mul(u2T[:, :], u2T[:, :], u2T[:, :])

        # S1 = sum(u1), S2 = sum(u2) via ones-matmul + free-dim reduce
        s1v = psr.tile([1, 9], F32, tag=red_tag)
        nc.tensor.matmul(s1v[:, :], ones_k[:, :], u1[0:CIN, :], start=True, stop=True)
        s2v = psr.tile([1, COUT], F32, tag=red_tag)
        nc.tensor.matmul(s2v[:, :], ones_k[:, :], u2T[0:CIN, :], start=True, stop=True)
        s1 = wp.tile([1, 1], F32, tag="s1")
        nc.vector.tensor_reduce(
            s1[:, :], s1v[:, :], axis=mybir.AxisListType.X, op=mybir.AluOpType.add
        )
        s2 = wp.tile([1, 1], F32, tag="s2")
        nc.vector.tensor_reduce(
            s2[:, :], s2v[:, :], axis=mybir.AxisListType.X, op=mybir.AluOpType.add
        )
        inv = wp.tile([1, 1], F32, tag="inv")
        nc.vector.tensor_mul(inv[:, :], s1[:, :], s2[:, :])
        nc.vector.reciprocal(inv[:, :], inv[:, :])
        # broadcast 1/(S1*S2) to all 128 partitions
        invb_ps = psr.tile([2 * CIN, 1], F32, tag=red_tag)
        nc.tensor.matmul(invb_ps[:, :], ones_m[:, :], inv[:, :], start=True, stop=True)
        invb = wp.tile([2 * CIN, 1], F32, tag="invb")
        nc.vector.tensor_copy(invb[:, :], invb_ps[:, :])
        # u2T_n = u2T / (S1*S2): both normalizations in one fold
        u2Tn = wp.tile([2 * CIN, COUT], F32, tag="u2Tn")
        nc.vector.tensor_scalar(
            u2Tn[:, :], u2T[:, :], invb[:, 0:1], None, mybir.AluOpType.mult
        )

        # fused weight tiles
        #  hsplit: 9 column blocks, block k = w2T*w1[:,k], same both halves
        #  dual:   6 column blocks with per-half k (see _mm_dual):
        #          slot:   0     1     2     3     4     5
        #          lower:  k0    k1    k2    k3    k4    k5
        #          upper:  k6    k7    k8    k5    -     -
        if SCHEME == "dual":
            half_ks = ((0, 1, 2, 3, 4, 5), (6, 7, 8, 5))
            n_blocks = 6
        else:
            half_ks = (tuple(range(9)), tuple(range(9)))
            n_blocks = 9
        wf_tiles = {}
        for s, base, w1s in (("x", w2xT_s, w1x_s), ("y", w2yT_s, w1y_s), ("n", u2Tn, u1)):
            wf = wp.tile([2 * CIN, n_blocks * COUT], mdt, tag=f"wf{s}")
            for half in (0, 1):
                p0, p1 = half * CIN, (half + 1) * CIN
                for slot, k in enumerate(half_ks[half]):
                    nc.vector.tensor_scalar(
                        wf[p0:p1, slot * COUT : (slot + 1) * COUT],
                        base[p0:p1, :],
                        w1s[p0:p1, k : k + 1],
                        None,
                        mybir.AluOpType.mult,
                    )
            wf_tiles[s] = wf

        # ---- main compute ------------------------------------------------
        def hpair_branch(b, s, needs_log):
            """hpair: per 8-row band, interleave the lower-half tile (out rows
            8*tpl..) with the upper-half tile (out rows 32+8*tpl..) MM-by-MM so
            consecutive matmuls land on disjoint PE row groups and overlap.
            Full branch output staged in SBUF, one DMA per branch."""
            wf = wf_tiles[s]
            img = imgp.tile([2 * CIN, img_rows, WP], mdt, tag="img")
            nc.sync.dma_start(out=img[0:CIN], in_=xin_d[b, 0])
            nc.sync.dma_start(out=img[CIN : 2 * CIN], in_=xin_d[b, 1])
            if needs_log:
                nc.scalar.activation(
                    img[:, :, :],
                    img[:, :, :],
                    mybir.ActivationFunctionType.Ln,
                    bias=eps_b[:, 0:1],
                )
            ot = obp.tile([COUT, H, W], _out_dt(), tag="ot")
            for tpl in range(4):
                psL = psp.tile([COUT, 8, W], F32, tag="ps")
                psU = psp.tile([COUT, 8, W], F32, tag="ps")
                if PROBE != "no_mm":
                    for k in range(9):
                        dh, dw = k // 3 - 1, k % 3 - 1
                        c0 = 1 + dw
                        rL = 8 * tpl + 1 + dh
                        rU = rL + 1
                        nc.tensor.matmul(
                            psL[:, :, :],
                            _wfk(wf, k, 0),
                            img[0:CIN, rL : rL + 8, c0 : c0 + W],
                            start=(k == 0),
                            stop=(k == 8),
                        )
                        nc.tensor.matmul(
                            psU[:, :, :],
                            _wfk(wf, k, 1),
                            img[CIN : 2 * CIN, rU : rU + 8, c0 : c0 + W],
                            start=(k == 0),
                            stop=(k == 8),
                        )
                if PROBE == "no_out":
                    continue
                for half, ps in ((0, psL), (1, psU)):
                    dst = ot[:, 8 * tpl + 32 * half : 8 * tpl + 32 * half + 8, :]
                    if needs_log:
                        nc.scalar.activation(
                            dst, ps[:, :, :], mybir.ActivationFunctionType.Exp,
                            bias=zero_b[:, 0:1],
                        )
                    elif half == 0:
                        nc.vector.tensor_copy(dst, ps[:, :, :])
                    else:
                        nc.scalar.activation(
                            dst, ps[:, :, :], mybir.ActivationFunctionType.Copy
                        )
            if PROBE != "no_out":
                nc.sync.dma_start(out=out_d[b], in_=ot[:, :, :])

        def houter_branch(b, s, needs_log):
            """k-outer: all 8 PSUM banks accumulate one branch; per k-offset
            the weight block stays stationary across the 4 row-bands of its
            half, with lower/upper halves interleaved MM-by-MM."""
            wf = wf_tiles[s]
            img = imgp.tile([2 * CIN, img_rows, WP], mdt, tag="img")
            nc.sync.dma_start(out=img[0:CIN], in_=xin_d[b, 0])
            nc.sync.dma_start(out=img[CIN : 2 * CIN], in_=xin_d[b, 1])
            if needs_log:
                nc.scalar.activation(
                    img[:, :, :],
                    img[:, :, :],
                    mybir.ActivationFunctionType.Ln,
                    bias=eps_b[:, 0:1],
                )
            ot = obp.tile([COUT, H, W], _out_dt(), tag="ot")
            pss = [
                psp.tile([COUT, 8, W], F32, tag="ps", name=f"ps{i}")
                for i in range(8)
            ]
            if PROBE != "no_mm":
                for k in range(9):
                    dh, dw = k // 3 - 1, k % 3 - 1
                    c0 = 1 + dw
                    for tpl in range(4):
                        rL = 8 * tpl + 1 + dh
                        nc.tensor.matmul(
                            pss[2 * tpl][:, :, :],
                            _wfk(wf, k, 0),
                            img[0:CIN, rL : rL + 8, c0 : c0 + W],
                            start=(k == 0),
                            stop=(k == 8),
                        )
                        nc.tensor.matmul(
                            pss[2 * tpl + 1][:, :, :],
                            _wfk(wf, k, 1),
                            img[CIN : 2 * CIN, rL + 1 : rL + 9, c0 : c0 + W],
                            start=(k == 0),
                            stop=(k == 8),
                        )
            if PROBE == "no_out":
                return
            for tpl in range(4):
                for half in (0, 1):
                    ps = pss[2 * tpl + half]
                    h0 = 8 * tpl + 32 * half
                    dst = ot[:, h0 : h0 + 8, :]
                    if needs_log:
                        nc.scalar.activation(
                            dst, ps[:, :, :], mybir.ActivationFunctionType.Exp,
                            bias=zero_b[:, 0:1],
                        )
                    elif half == 0:
                        nc.vector.tensor_copy(dst, ps[:, :, :])
                    else:
                        nc.scalar.activation(
                            dst, ps[:, :, :], mybir.ActivationFunctionType.Copy
                        )
            nc.sync.dma_start(out=out_d[b], in_=ot[:, :, :])

        # weight-register WAR fences for hldw: per half, the matmuls still
        # consuming the currently-loaded weights; the next ldweights for that
        # half must not be scheduled above them.
        _ldw_fence = {0: [], 1: []}

        def hldw_branch(b, s, needs_log):
            """k-outer with explicit LDWEIGHTS: each (k, half) weight block is
            loaded once and consumed by 4 non-self-loading matmuls (the 4
            row-bands of its half), halves interleaved MM-by-MM."""
            from concourse.tile_rust import add_dep_helper

            wf = wf_tiles[s]
            img = imgp.tile([2 * CIN, img_rows, WP], mdt, tag="img")
            nc.sync.dma_start(out=img[0:CIN], in_=xin_d[b, 0])
            nc.sync.dma_start(out=img[CIN : 2 * CIN], in_=xin_d[b, 1])
            if needs_log:
                nc.scalar.activation(
                    img[:, :, :],
                    img[:, :, :],
                    mybir.ActivationFunctionType.Ln,
                    bias=eps_b[:, 0:1],
                )
            ot = obp.tile([COUT, H, W], _out_dt(), tag="ot")
            pss = [
                psp.tile([COUT, 8, W], F32, tag="ps", name=f"ps{i}")
                for i in range(8)
            ]
            if PROBE != "no_mm":
                for k in range(9):
                    dh, dw = k // 3 - 1, k % 3 - 1
                    c0 = 1 + dw
                    ldw = {}
                    for half in (0, 1):
                        ldw[half] = nc.tensor.ldweights(
                            _wfk(wf, k, half), tile_position=(64 * half, 0)
                        )
                        for m in _ldw_fence[half]:
                            add_dep_helper(
                                ldw[half].ins, m, reason="weight WAR fence"
                            )
                        _ldw_fence[half] = []
                    for tpl in range(4):
                        rL = 8 * tpl + 1 + dh
                        for half, r0 in ((0, rL), (1, rL + 1)):
                            p0 = half * CIN
                            mm = nc.tensor.matmul(
                                pss[2 * tpl + half][:, :, :],
                                _wfk(wf, k, half),
                                img[p0 : p0 + CIN, r0 : r0 + 8, c0 : c0 + W],
                                start=(k == 0),
                                stop=(k == 8),
                            )
                            mm.ins.ldweights = False
                            add_dep_helper(
                                mm.ins, ldw[half].ins, reason="mm after its ldw"
                            )
                            _ldw_fence[half].append(mm.ins)
            if PROBE == "no_out":
                return
            for tpl in range(4):
                for half in (0, 1):
                    ps = pss[2 * tpl + half]
                    h0 = 8 * tpl + 32 * half
                    dst = ot[:, h0 : h0 + 8, :]
                    if needs_log:
                        nc.scalar.activation(
                            dst, ps[:, :, :], mybir.ActivationFunctionType.Exp,
                            bias=zero_b[:, 0:1],
                        )
                    elif half == 0:
                        nc.vector.tensor_copy(dst, ps[:, :, :])
                    else:
                        nc.scalar.activation(
                            dst, ps[:, :, :], mybir.ActivationFunctionType.Copy
                        )
            nc.sync.dma_start(out=out_d[b], in_=ot[:, :, :])

        def main_body():
            for b, s, needs_log, evac in BRANCHES:
                if SCHEME == "hpair":
                    hpair_branch(b, s, needs_log)
                    continue
                if SCHEME == "houter":
                    houter_branch(b, s, needs_log)
                    continue
                if SCHEME == "hldw":
                    hldw_branch(b, s, needs_log)
                    continue
                wf = wf_tiles[s]
                img = imgp.tile([2 * CIN, img_rows, WP], mdt, tag="img")
                nc.sync.dma_start(out=img[0:CIN], in_=xin_d[b, 0])
                nc.sync.dma_start(out=img[CIN : 2 * CIN], in_=xin_d[b, 1])
                if needs_log:
                    nc.scalar.activation(
                        img[:, :, :],
                        img[:, :, :],
                        mybir.ActivationFunctionType.Ln,
                        bias=eps_b[:, 0:1],
                    )
                for tp in range(8):
                    ps = psp.tile([COUT, 8, W], F32, tag="ps")
                    if PROBE != "no_mm":
                        if SCHEME == "dual":
                            _mm_dual(nc, ps, wf, img, tp)
                        else:
                            _mm_hsplit(nc, ps, wf, img, tp)
                    if PROBE == "no_out":
                        continue
                    ot = obp.tile([COUT, 8, W], _out_dt(), tag="ot")
                    h0 = 8 * tp
                    if needs_log:
                        nc.scalar.activation(
                            ot[:, :, :],
                            ps[:, :, :],
                            mybir.ActivationFunctionType.Exp,
                            bias=zero_b[:, 0:1],
                        )
                    elif evac == "v":
                        nc.vector.tensor_copy(ot[:, :, :], ps[:, :, :])
                    else:
                        nc.scalar.activation(
                            ot[:, :, :], ps[:, :, :], mybir.ActivationFunctionType.Copy
                        )
                    nc.sync.dma_start(out=out_d[b, :, h0 : h0 + 8, :], in_=ot[:, :, :])

        if LOOP_ITERS:
            with tc.For_i(0, LOOP_ITERS, 1):
                main_body()
        else:
            main_body()


def _wfk(wf, k, half):
    p0, p1 = half * CIN, (half + 1) * CIN
    return wf[p0:p1, k * COUT : (k + 1) * COUT]


def _mm_dual(nc, ps, wf, img, tp):
    """out rows 8*tp..8*tp+7 from dual-copy image: partitions 0..63 hold the
    padded image A (rows 0..65), partitions 64..127 hold B with B[r]=A[r+2].

    6 matmuls per tile: 3x K=128 (offset pairs (-1,dw)+(+1,dw)), then the
    dh=0 row as K=64 matmuls -- (0,-1) on the lower row group packed with
    (0,+1) on the upper row group (concurrent), plus (0,0) on the lower."""
    h0 = 8 * tp
    n_mm = 6
    idx = [0]

    def step(lhsT, rhs):
        nc.tensor.matmul(
            ps[:, :, :], lhsT, rhs, start=(idx[0] == 0), stop=(idx[0] == n_mm - 1)
        )
        idx[0] += 1

    for dw in (-1, 0, 1):  # slots 0..2: K=128, lower k=dw+1, upper k=7+dw
        step(
            wf[:, (dw + 1) * COUT : (dw + 2) * COUT],
            img[:, h0 : h0 + 8, 1 + dw : 1 + dw + W],
        )
    # (0,-1) lower (slot3 low) ++ (0,+1) upper (slot3 high, B[h0-1]=A[h0+1])
    step(wf[0:CIN, 3 * COUT : 4 * COUT], img[0:CIN, h0 + 1 : h0 + 9, 0:W])
    if DH0_UPPER_PACK and tp > 0:
        step(
            wf[CIN : 2 * CIN, 3 * COUT : 4 * COUT],
            img[CIN : 2 * CIN, h0 - 1 : h0 + 7, 2 : 2 + W],
        )
    else:  # B row -1 unavailable (tp=0) or packing disabled: lower, slot 5
        step(wf[0:CIN, 5 * COUT : 6 * COUT], img[0:CIN, h0 + 1 : h0 + 9, 2 : 2 + W])
    # (0,0) lower (slot4 low)
    step(wf[0:CIN, 4 * COUT : 5 * COUT], img[0:CIN, h0 + 1 : h0 + 9, 1 : 1 + W])


def _mm_hsplit(nc, ps, wf, img, tp):
    """hsplit scheme: tile tp covers out rows 8*tp..+7; lower tiles (tp<4)
    read partitions 0..63, upper tiles read 64..127."""
    half = 0 if tp < 4 else 1
    p0, p1 = half * CIN, (half + 1) * CIN
    tpl = tp % 4
    for k in range(9):
        dh, dw = k // 3 - 1, k % 3 - 1
        r = 8 * tpl + 1 + dh + half  # lower: pad row - 0; upper: pad row - 31
        c0 = 1 + dw
        nc.tensor.matmul(
            ps[:, :, :],
            _wfk(wf, k, half),
            img[p0:p1, r : r + 8, c0 : c0 + W],
            start=(k == 0),
            stop=(k == 8),
        )


def build_program():
    key = (MM_DTYPE, SCHEME, OUT_DTYPE, LOOP_ITERS, DH0_UPPER_PACK, PROBE, LDW_OPT)
    if key in _PROG_CACHE:
        return _PROG_CACHE[key]
    img_rows = HP if SCHEME == "dual" else HS_ROWS
    nc = bacc.Bacc("TRN2", target_bir_lowering=False, debug=False)
    xin_d = nc.dram_tensor(
        "xin", [NBR, 2, CIN, img_rows, WP], _mm_dt(), kind="ExternalInput"
    ).ap()
    w1x_d = nc.dram_tensor("w1x", [CIN, 9], F32, kind="ExternalInput").ap()
    w1y_d = nc.dram_tensor("w1y", [CIN, 9], F32, kind="ExternalInput").ap()
    w2xT_d = nc.dram_tensor("w2xT", [CIN, COUT], F32, kind="ExternalInput").ap()
    w2yT_d = nc.dram_tensor("w2yT", [CIN, COUT], F32, kind="ExternalInput").ap()
    out_d = nc.dram_tensor(
        "out", [NBR, COUT, H, W], _out_dt(), kind="ExternalOutput"
    ).ap()
    with tile.TileContext(nc) as tc:
        _emit(nc, tc, xin_d, w1x_d, w1y_d, w2xT_d, w2yT_d, out_d)
    nc.compile()
    _PROG_CACHE[key] = nc
    return nc


def marshal_inputs(x, w1x, w1y, w2x, w2y):
    """Host-side data marshaling: shard over batch, zero-pad, build the
    per-partition-half copies for the selected scheme."""
    ndt = _np_in_dt()
    x = np.asarray(x, dtype=np.float32)
    xp = np.zeros((B, NBR, CIN, HP, WP), np.float32)
    xp[:, :, :, 1 : H + 1, 1 : W + 1] = x
    if SCHEME == "dual":
        xin = np.zeros((B, NBR, 2, CIN, HP, WP), ndt)
        xin[:, :, 0] = xp.astype(ndt)
        xin[:, :, 1, :, 0 : HP - 2, :] = xp[:, :, :, 2:HP, :].astype(ndt)
    else:
        xin = np.empty((B, NBR, 2, CIN, HS_ROWS, WP), ndt)
        xin[:, :, 0] = xp[:, :, :, 0:HS_ROWS, :].astype(ndt)
        xin[:, :, 1] = xp[:, :, :, HP - HS_ROWS : HP, :].astype(ndt)
    w2xT = np.ascontiguousarray(np.asarray(w2x, np.float32).T)
    w2yT = np.ascontiguousarray(np.asarray(w2y, np.float32).T)
    w1x = np.ascontiguousarray(w1x, np.float32)
    w1y = np.ascontiguousarray(w1y, np.float32)
    return [
        {
            "xin": np.ascontiguousarray(xin[i]),
            "w1x": w1x,
            "w1y": w1y,
            "w2xT": w2xT,
            "w2yT": w2yT,
        }
        for i in range(B)
    ]


def kernel(x, w1x, w1y, w2x, w2y):
    global LAST_EXEC_TIME_NS, LAST_RESULTS
    nc = build_program()
    in_maps = marshal_inputs(x, w1x, w1y, w2x, w2y)
    res = bass_utils.run_bass_kernel_spmd(
        nc, in_maps, list(range(N_CORES)), trace=TRACE
    )
    LAST_EXEC_TIME_NS = res.exec_time_ns
    LAST_RESULTS = res
    out = np.stack(
        [np.asarray(res.results[i]["out"], np.float32) for i in range(N_CORES)], axis=0
    )
    return out



# revision 30
# speedup vs baseline: 2.4907x; 2.4907x over previous
"""Trainium2 Bass kernel for nn_ComplexConv2Deffangle4Dxy.

Reference math (per batch b, branch br):
    out[br] = pointwise(w2, depthwise3x3(w1, img[br]))   with zero padding P=1
      br=0 (rot): weights (w1n, w2n) where wn = (wx+wy)^2 / sum((wx+wy)^2)
      br=1 (abs): log-domain: exp(branch(log(img + EPS), w1n, w2n))
      br=2 (x):   weights (w1x, w2x)
      br=3 (y):   weights (w1y, w2y)

Kernel strategy (per NeuronCore, data-parallel over batch B=8 -> 8 cores):
  Fuse depthwise+pointwise into a single 3x3 conv whose weights are the
  outer product  Wf[o, c, k] = w2[o, c] * w1[c, k], computed as
  PSUM-accumulated matmuls over the 9 kernel offsets with
  lhsT = fused weights (K=Cin, M=Cout=128) and rhs = shifted image views.
  Images are zero-padded on the host (pure marshaling) so every shifted
  view is a plain strided AP with no boundary special cases; for the abs
  branch Ln(x*1+EPS) maps the zero padding to log(EPS), exactly matching
  the reference's pad-then-log order.  Weight normalization for the
  rot/abs branches is computed on device (sum via ones-matmul, reciprocal
  on DVE, scale folded into the fused conv weights).

  Scheme "hpair" (default, fastest measured): partitions 0..63 hold padded
  image rows 0..34, partitions 64..127 hold rows 31..65.  Per 8-row output
  band pair, the lower-half tile (rows 8t..) and upper-half tile (rows
  32+8t..) are emitted matmul-by-matmul interleaved so consecutive K=64
  matmuls land on disjoint PE row groups and overlap (measured ~175 ns/MM
  vs ~508 ns/MM for a serial same-row-group chain; the 2.4 GHz stream
  floor is 213 ns).  bf16 operands (max rel err vs fp32 reference 4.4e-3,
  gate 2e-2), bf16 outputs (halves out-DMA bytes; host upcasts), branches
  ordered (x, abs, y, rot) so the ACT-heavy Exp evacuation overlaps x/y
  matmuls, and each band's output DMA is issued right after its PSUM
  evacuation so the kernel tail is one band, not one branch.

  Measured on-HW per-iteration (device-side For_i repeat, two-point
  wall-clock): baseline f32r/hsplit 133 us -> hpair/bf16 48.6 us.

  Findings from HW microbenchmarks (micro.py), kept for future tuning:
    - every nc.tensor.matmul is compiled into InstLdweights + non-self-
      loading InstMatmult; _dedupe_ldweights() can safely drop reloads of
      the already-resident weights (verified bit-correct on HW) but weight
      reuse does NOT speed up the stream -- the ~177 ns/MM pair floor is
      LDW-independent (pair_r36_b8 181 vs pair_b8 177 ns/MM).
    - serial same-row-group chains cost ~508-584 ns/MM regardless of LDW
      count; alternating row halves is the single biggest lever.
    - K=128 matmuls run at ~281 ns/MM with 8-bank cycling (140 ns per
      contracted offset) but the odd dh=0 offset row cannot be paired
      (mixing K=64 upper-half matmuls with K=128 in one PSUM accumulation
      group crashes TRN2), so a dual-style scheme does not win.
    - k-outer orderings (8 live PSUM accumulators) measured slower than
      band-pair ping-pong despite 4x weight reuse (houter 58.5 us).

  Older schemes kept for A/B: "dual", "hsplit" (see git of prior session),
  "houter", "hldw".
"""

import sys

for _p in ("/opt/trn_rl_repo",):
    if _p not in sys.path:
        sys.path.insert(0, _p)

import ml_dtypes
import numpy as np

import concourse.bacc as bacc
import concourse.mybir as mybir
import concourse.tile as tile
from concourse import bass_utils

F32 = mybir.dt.float32
F32R = mybir.dt.float32r
BF16 = mybir.dt.bfloat16

EPS = 1e-6
N_CORES = 8
B, NBR, CIN, COUT, H, W = 8, 4, 64, 128, 64, 64
HP, WP = H + 2, W + 2          # host-padded image
HS_ROWS = 35                   # hsplit: padded rows per partition half

# matmul input dtype: "f32r" | "f32" | "bf16"
MM_DTYPE = "bf16"
SCHEME = "hpair"               # "dual" | "hsplit" | "hpair" | "houter" | "hldw"
# output dtype on device: "f32" | "bf16" (bf16 halves the out-DMA traffic;
# host upcasts -- adds <=0.4% rounding, gate is 2e-2)
OUT_DTYPE = "bf16"
# Packing (0,+1) onto the upper PE row group (K=64 at base_partition 64)
# mixed with K=128 matmuls in the same PSUM accumulation group crashes at
# runtime on TRN2 hardware -- keep disabled.
DH0_UPPER_PACK = False
DEDUP_LDW = True               # post-compile: drop InstLdweights that reload
                               # the row-group's already-loaded weights
LOOP_ITERS = None              # benchmarking: device-side repeat count
PROBE = ""                     # "" | "no_out" (skip evac+out-DMA) | "no_mm"
TRACE = False
LAST_EXEC_TIME_NS = None
LAST_RESULTS = None

_PROG_CACHE = {}

# walrus's --enable-ldw-opt crashes at runtime on TRN2 (tested); the flag is
# kept only so older bench configs keying on it still build.
LDW_OPT = False

BRANCHES = (  # (branch index, weight set, log-domain?, evac engine)
    # abs (Exp evac, ACT-heavy) goes second so its activations overlap the
    # x/y matmul streams and the kernel tail ends on a cheap copy-evac branch
    (2, "x", False, "v"),
    (1, "n", True, "a"),
    (3, "y", False, "a"),
    (0, "n", False, "v"),
)


def _mm_dt():
    return {"f32r": F32R, "f32": F32, "bf16": BF16}[MM_DTYPE]


def _np_in_dt():
    return ml_dtypes.bfloat16 if MM_DTYPE == "bf16" else np.float32


def _out_dt():
    return BF16 if OUT_DTYPE == "bf16" else F32


def _emit(nc, tc, xin_d, w1x_d, w1y_d, w2xT_d, w2yT_d, out_d):
    import contextlib

    mdt = _mm_dt()
    img_rows = HP if SCHEME == "dual" else HS_ROWS
    # houter keeps 8 PSUM accumulators live for a whole branch -> needs all 8
    # banks in one pool; weight-prep reductions then share psp (they finish
    # before the main loop starts).
    psp_bufs = 8 if SCHEME in ("houter", "hldw") else 6
    obp_bufs = 2 if SCHEME in ("hpair", "houter", "hldw") else 6
    with contextlib.ExitStack() as _stack:
        wp = _stack.enter_context(tc.tile_pool(name="wp", bufs=1))
        imgp = _stack.enter_context(tc.tile_pool(name="imgp", bufs=2))
        psp = _stack.enter_context(tc.tile_pool(name="psp", bufs=psp_bufs, space="PSUM"))
        obp = _stack.enter_context(tc.tile_pool(name="obp", bufs=obp_bufs))
        if SCHEME in ("houter", "hldw"):
            psr, red_tag = psp, "ps"
        else:
            psr = _stack.enter_context(tc.tile_pool(name="psr", bufs=2, space="PSUM"))
            red_tag = "red"
        # ---- weight prep -------------------------------------------------
        # All weight/source tiles replicated into both partition halves so
        # per-half fused tiles can be built with partition-local DVE ops.
        w1x_s = wp.tile([2 * CIN, 9], F32, tag="w1x")
        w1y_s = wp.tile([2 * CIN, 9], F32, tag="w1y")
        w2xT_s = wp.tile([2 * CIN, COUT], F32, tag="w2xT")
        w2yT_s = wp.tile([2 * CIN, COUT], F32, tag="w2yT")
        for t, d in (
            (w1x_s, w1x_d),
            (w1y_s, w1y_d),
            (w2xT_s, w2xT_d),
            (w2yT_s, w2yT_d),
        ):
            nc.sync.dma_start(out=t[0:CIN], in_=d)
            nc.sync.dma_start(out=t[CIN : 2 * CIN], in_=d)

        ones_k = wp.tile([CIN, 1], F32, tag="ones_k")
        nc.vector.memset(ones_k[:, :], 1.0)
        ones_m = wp.tile([1, 2 * CIN], F32, tag="ones_m")
        nc.vector.memset(ones_m[:, :], 1.0)
        eps_b = wp.tile([2 * CIN, 1], F32, tag="eps_b")
        nc.vector.memset(eps_b[:, :], float(EPS))
        zero_b = wp.tile([COUT, 1], F32, tag="zero_b")
        nc.vector.memset(zero_b[:, :], 0.0)

        # u1 = (w1x + w1y)^2, u2T = ((w2x + w2y)^2)^T  (both partition halves)
        u1 = wp.tile([2 * CIN, 9], F32, tag="u1")
        nc.vector.tensor_add(u1[:, :], w1x_s[:, :], w1y_s[:, :])
        nc.vector.tensor_mul(u1[:, :], u1[:, :], u1[:, :])
        u2T = wp.tile([2 * CIN, COUT], F32, tag="u2T")
        nc.vector.tensor_add(u2T[:, :], w2xT_s[:, :], w2yT_s[:, :])
        nc.vector.tensor_mul(u2T[:, :], u2T[:, :], u2T[:, :])

        # S1 = sum(u1), S2 = sum(u2) via ones-matmul + free-dim reduce
        s1v = psr.tile([1, 9], F32, tag=red_tag)
        nc.tensor.matmul(s1v[:, :], ones_k[:, :], u1[0:CIN, :], start=True, stop=True)
        s2v = psr.tile([1, COUT], F32, tag=red_tag)
        nc.tensor.matmul(s2v[:, :], ones_k[:, :], u2T[0:CIN, :], start=True, stop=True)
        s1 = wp.tile([1, 1], F32, tag="s1")
        nc.vector.tensor_reduce(
            s1[:, :], s1v[:, :], axis=mybir.AxisListType.X, op=mybir.AluOpType.add
        )
        s2 = wp.tile([1, 1], F32, tag="s2")
        nc.vector.tensor_reduce(
            s2[:, :], s2v[:, :], axis=mybir.AxisListType.X, op=mybir.AluOpType.add
        )
        inv = wp.tile([1, 1], F32, tag="inv")
        nc.vector.tensor_mul(inv[:, :], s1[:, :], s2[:, :])
        nc.vector.reciprocal(inv[:, :], inv[:, :])
        # broadcast 1/(S1*S2) to all 128 partitions
        invb_ps = psr.tile([2 * CIN, 1], F32, tag=red_tag)
        nc.tensor.matmul(invb_ps[:, :], ones_m[:, :], inv[:, :], start=True, stop=True)
        invb = wp.tile([2 * CIN, 1], F32, tag="invb")
        nc.vector.tensor_copy(invb[:, :], invb_ps[:, :])
        # u2T_n = u2T / (S1*S2): both normalizations in one fold
        u2Tn = wp.tile([2 * CIN, COUT], F32, tag="u2Tn")
        nc.vector.tensor_scalar(
            u2Tn[:, :], u2T[:, :], invb[:, 0:1], None, mybir.AluOpType.mult
        )

        # fused weight tiles
        #  hsplit: 9 column blocks, block k = w2T*w1[:,k], same both halves
        #  dual:   6 column blocks with per-half k (see _mm_dual):
        #          slot:   0     1     2     3     4     5
        #          lower:  k0    k1    k2    k3    k4    k5
        #          upper:  k6    k7    k8    k5    -     -
        if SCHEME == "dual":
            half_ks = ((0, 1, 2, 3, 4, 5), (6, 7, 8, 5))
            n_blocks = 6
        else:
            half_ks = (tuple(range(9)), tuple(range(9)))
            n_blocks = 9
        wf_tiles = {}
        for s, base, w1s in (("x", w2xT_s, w1x_s), ("y", w2yT_s, w1y_s), ("n", u2Tn, u1)):
            wf = wp.tile([2 * CIN, n_blocks * COUT], mdt, tag=f"wf{s}")
            for half in (0, 1):
                p0, p1 = half * CIN, (half + 1) * CIN
                for slot, k in enumerate(half_ks[half]):
                    nc.vector.tensor_scalar(
                        wf[p0:p1, slot * COUT : (slot + 1) * COUT],
                        base[p0:p1, :],
                        w1s[p0:p1, k : k + 1],
                        None,
                        mybir.AluOpType.mult,
                    )
            wf_tiles[s] = wf

        # ---- main compute ------------------------------------------------
        def hpair_branch(b, s, needs_log):
            """hpair: per 8-row band, interleave the lower-half tile (out rows
            8*tpl..) with the upper-half tile (out rows 32+8*tpl..) MM-by-MM so
            consecutive matmuls land on disjoint PE row groups and overlap.
            Full branch output staged in SBUF, one DMA per branch."""
            wf = wf_tiles[s]
            img = imgp.tile([2 * CIN, img_rows, WP], mdt, tag="img")
            nc.sync.dma_start(out=img[0:CIN], in_=xin_d[b, 0])
            nc.sync.dma_start(out=img[CIN : 2 * CIN], in_=xin_d[b, 1])
            if needs_log:
                nc.scalar.activation(
                    img[:, :, :],
                    img[:, :, :],
                    mybir.ActivationFunctionType.Ln,
                    bias=eps_b[:, 0:1],
                )
            ot = obp.tile([COUT, H, W], _out_dt(), tag="ot")
            for tpl in range(4):
                psL = psp.tile([COUT, 8, W], F32, tag="ps")
                psU = psp.tile([COUT, 8, W], F32, tag="ps")
                if PROBE != "no_mm":
                    for k in range(9):
                        dh, dw = k // 3 - 1, k % 3 - 1
                        c0 = 1 + dw
                        rL = 8 * tpl + 1 + dh
                        rU = rL + 1
                        nc.tensor.matmul(
                            psL[:, :, :],
                            _wfk(wf, k, 0),
                            img[0:CIN, rL : rL + 8, c0 : c0 + W],
                            start=(k == 0),
                            stop=(k == 8),
                        )
                        nc.tensor.matmul(
                            psU[:, :, :],
                            _wfk(wf, k, 1),
                            img[CIN : 2 * CIN, rU : rU + 8, c0 : c0 + W],
                            start=(k == 0),
                            stop=(k == 8),
                        )
                if PROBE == "no_out":
                    continue
                for half, ps in ((0, psL), (1, psU)):
                    h0 = 8 * tpl + 32 * half
                    dst = ot[:, h0 : h0 + 8, :]
                    if needs_log:
                        nc.scalar.activation(
                            dst, ps[:, :, :], mybir.ActivationFunctionType.Exp,
                            bias=zero_b[:, 0:1],
                        )
                    elif half == 0:
                        nc.vector.tensor_copy(dst, ps[:, :, :])
                    else:
                        nc.scalar.activation(
                            dst, ps[:, :, :], mybir.ActivationFunctionType.Copy
                        )
                    # chunked out-DMA: ship each band as soon as it is evac'd
                    # so the kernel tail is one band, not one branch
                    nc.sync.dma_start(
                        out=out_d[b, :, h0 : h0 + 8, :], in_=dst
                    )

        def houter_branch(b, s, needs_log):
            """k-outer: all 8 PSUM banks accumulate one branch; per k-offset
            the weight block stays stationary across the 4 row-bands of its
            half, with lower/upper halves interleaved MM-by-MM."""
            wf = wf_tiles[s]
            img = imgp.tile([2 * CIN, img_rows, WP], mdt, tag="img")
            nc.sync.dma_start(out=img[0:CIN], in_=xin_d[b, 0])
            nc.sync.dma_start(out=img[CIN : 2 * CIN], in_=xin_d[b, 1])
            if needs_log:
                nc.scalar.activation(
                    img[:, :, :],
                    img[:, :, :],
                    mybir.ActivationFunctionType.Ln,
                    bias=eps_b[:, 0:1],
                )
            ot = obp.tile([COUT, H, W], _out_dt(), tag="ot")
            pss = [
                psp.tile([COUT, 8, W], F32, tag="ps", name=f"ps{i}")
                for i in range(8)
            ]

            def evac(tpl, half):
                ps = pss[2 * tpl + half]
                h0 = 8 * tpl + 32 * half
                dst = ot[:, h0 : h0 + 8, :]
                if needs_log:
                    nc.scalar.activation(
                        dst, ps[:, :, :], mybir.ActivationFunctionType.Exp,
                        bias=zero_b[:, 0:1],
                    )
                elif half == 0:
                    nc.vector.tensor_copy(dst, ps[:, :, :])
                else:
                    nc.scalar.activation(
                        dst, ps[:, :, :], mybir.ActivationFunctionType.Copy
                    )

            def mm(k, tpl, half):
                dh, dw = k // 3 - 1, k % 3 - 1
                r0 = 8 * tpl + 1 + dh + half
                p0 = half * CIN
                nc.tensor.matmul(
                    pss[2 * tpl + half][:, :, :],
                    _wfk(wf, k, half),
                    img[p0 : p0 + CIN, r0 : r0 + 8, 1 + dw : 1 + dw + W],
                    start=(k == 0),
                    stop=(k == 8),
                )

            if PROBE != "no_mm":
                for k in range(8):
                    for tpl in range(4):
                        mm(k, tpl, 0)
                        mm(k, tpl, 1)
                # last k-round: evac each band right after its closing matmul
                # so PSUM drains overlap the round instead of bursting at the
                # branch boundary
                for tpl in range(4):
                    for half in (0, 1):
                        mm(8, tpl, half)
                        if PROBE != "no_out":
                            evac(tpl, half)
            elif PROBE != "no_out":
                for tpl in range(4):
                    for half in (0, 1):
                        evac(tpl, half)
            if PROBE == "no_out":
                return
            nc.sync.dma_start(out=out_d[b], in_=ot[:, :, :])

        # weight-register WAR fences for hldw: per half, the matmuls still
        # consuming the currently-loaded weights; the next ldweights for that
        # half must not be scheduled above them.
        _ldw_fence = {0: [], 1: []}

        def hldw_branch(b, s, needs_log):
            """k-outer with explicit LDWEIGHTS: each (k, half) weight block is
            loaded once and consumed by 4 non-self-loading matmuls (the 4
            row-bands of its half), halves interleaved MM-by-MM."""
            from concourse.tile_rust import add_dep_helper

            wf = wf_tiles[s]
            img = imgp.tile([2 * CIN, img_rows, WP], mdt, tag="img")
            nc.sync.dma_start(out=img[0:CIN], in_=xin_d[b, 0])
            nc.sync.dma_start(out=img[CIN : 2 * CIN], in_=xin_d[b, 1])
            if needs_log:
                nc.scalar.activation(
                    img[:, :, :],
                    img[:, :, :],
                    mybir.ActivationFunctionType.Ln,
                    bias=eps_b[:, 0:1],
                )
            ot = obp.tile([COUT, H, W], _out_dt(), tag="ot")
            pss = [
                psp.tile([COUT, 8, W], F32, tag="ps", name=f"ps{i}")
                for i in range(8)
            ]
            if PROBE != "no_mm":
                for k in range(9):
                    dh, dw = k // 3 - 1, k % 3 - 1
                    c0 = 1 + dw
                    ldw = {}
                    for half in (0, 1):
                        ldw[half] = nc.tensor.ldweights(
                            _wfk(wf, k, half), tile_position=(64 * half, 0)
                        )
                        for m in _ldw_fence[half]:
                            add_dep_helper(
                                ldw[half].ins, m, reason="weight WAR fence"
                            )
                        _ldw_fence[half] = []
                    for tpl in range(4):
                        rL = 8 * tpl + 1 + dh
                        for half, r0 in ((0, rL), (1, rL + 1)):
                            p0 = half * CIN
                            mm = nc.tensor.matmul(
                                pss[2 * tpl + half][:, :, :],
                                _wfk(wf, k, half),
                                img[p0 : p0 + CIN, r0 : r0 + 8, c0 : c0 + W],
                                start=(k == 0),
                                stop=(k == 8),
                            )
                            mm.ins.ldweights = False
                            add_dep_helper(
                                mm.ins, ldw[half].ins, reason="mm after its ldw"
                            )
                            _ldw_fence[half].append(mm.ins)
            if PROBE == "no_out":
                return
            for tpl in range(4):
                for half in (0, 1):
                    ps = pss[2 * tpl + half]
                    h0 = 8 * tpl + 32 * half
                    dst = ot[:, h0 : h0 + 8, :]
                    if needs_log:
                        nc.scalar.activation(
                            dst, ps[:, :, :], mybir.ActivationFunctionType.Exp,
                            bias=zero_b[:, 0:1],
                        )
                    elif half == 0:
                        nc.vector.tensor_copy(dst, ps[:, :, :])
                    else:
                        nc.scalar.activation(
                            dst, ps[:, :, :], mybir.ActivationFunctionType.Copy
                        )
            nc.sync.dma_start(out=out_d[b], in_=ot[:, :, :])

        def main_body():
            for b, s, needs_log, evac in BRANCHES:
                if SCHEME == "hpair":
                    hpair_branch(b, s, needs_log)
                    continue
                if SCHEME == "houter":
                    houter_branch(b, s, needs_log)
                    continue
                if SCHEME == "hldw":
                    hldw_branch(b, s, needs_log)
                    continue
                wf = wf_tiles[s]
                img = imgp.tile([2 * CIN, img_rows, WP], mdt, tag="img")
                nc.sync.dma_start(out=img[0:CIN], in_=xin_d[b, 0])
                nc.sync.dma_start(out=img[CIN : 2 * CIN], in_=xin_d[b, 1])
                if needs_log:
                    nc.scalar.activation(
                        img[:, :, :],
                        img[:, :, :],
                        mybir.ActivationFunctionType.Ln,
                        bias=eps_b[:, 0:1],
                    )
                for tp in range(8):
                    ps = psp.tile([COUT, 8, W], F32, tag="ps")
                    if PROBE != "no_mm":
                        if SCHEME == "dual":
                            _mm_dual(nc, ps, wf, img, tp)
                        else:
                            _mm_hsplit(nc, ps, wf, img, tp)
                    if PROBE == "no_out":
                        continue
                    ot = obp.tile([COUT, 8, W], _out_dt(), tag="ot")
                    h0 = 8 * tp
                    if needs_log:
                        nc.scalar.activation(
                            ot[:, :, :],
                            ps[:, :, :],
                            mybir.ActivationFunctionType.Exp,
                            bias=zero_b[:, 0:1],
                        )
                    elif evac == "v":
                        nc.vector.tensor_copy(ot[:, :, :], ps[:, :, :])
                    else:
                        nc.scalar.activation(
                            ot[:, :, :], ps[:, :, :], mybir.ActivationFunctionType.Copy
                        )
                    nc.sync.dma_start(out=out_d[b, :, h0 : h0 + 8, :], in_=ot[:, :, :])

        if LOOP_ITERS:
            with tc.For_i(0, LOOP_ITERS, 1):
                main_body()
        else:
            main_body()


def _wfk(wf, k, half):
    p0, p1 = half * CIN, (half + 1) * CIN
    return wf[p0:p1, k * COUT : (k + 1) * COUT]


def _mm_dual(nc, ps, wf, img, tp):
    """out rows 8*tp..8*tp+7 from dual-copy image: partitions 0..63 hold the
    padded image A (rows 0..65), partitions 64..127 hold B with B[r]=A[r+2].

    6 matmuls per tile: 3x K=128 (offset pairs (-1,dw)+(+1,dw)), then the
    dh=0 row as K=64 matmuls -- (0,-1) on the lower row group packed with
    (0,+1) on the upper row group (concurrent), plus (0,0) on the lower."""
    h0 = 8 * tp
    n_mm = 6
    idx = [0]

    def step(lhsT, rhs):
        nc.tensor.matmul(
            ps[:, :, :], lhsT, rhs, start=(idx[0] == 0), stop=(idx[0] == n_mm - 1)
        )
        idx[0] += 1

    for dw in (-1, 0, 1):  # slots 0..2: K=128, lower k=dw+1, upper k=7+dw
        step(
            wf[:, (dw + 1) * COUT : (dw + 2) * COUT],
            img[:, h0 : h0 + 8, 1 + dw : 1 + dw + W],
        )
    # (0,-1) lower (slot3 low) ++ (0,+1) upper (slot3 high, B[h0-1]=A[h0+1])
    step(wf[0:CIN, 3 * COUT : 4 * COUT], img[0:CIN, h0 + 1 : h0 + 9, 0:W])
    if DH0_UPPER_PACK and tp > 0:
        step(
            wf[CIN : 2 * CIN, 3 * COUT : 4 * COUT],
            img[CIN : 2 * CIN, h0 - 1 : h0 + 7, 2 : 2 + W],
        )
    else:  # B row -1 unavailable (tp=0) or packing disabled: lower, slot 5
        step(wf[0:CIN, 5 * COUT : 6 * COUT], img[0:CIN, h0 + 1 : h0 + 9, 2 : 2 + W])
    # (0,0) lower (slot4 low)
    step(wf[0:CIN, 4 * COUT : 5 * COUT], img[0:CIN, h0 + 1 : h0 + 9, 1 : 1 + W])


def _mm_hsplit(nc, ps, wf, img, tp):
    """hsplit scheme: tile tp covers out rows 8*tp..+7; lower tiles (tp<4)
    read partitions 0..63, upper tiles read 64..127."""
    half = 0 if tp < 4 else 1
    p0, p1 = half * CIN, (half + 1) * CIN
    tpl = tp % 4
    for k in range(9):
        dh, dw = k // 3 - 1, k % 3 - 1
        r = 8 * tpl + 1 + dh + half  # lower: pad row - 0; upper: pad row - 31
        c0 = 1 + dw
        nc.tensor.matmul(
            ps[:, :, :],
            _wfk(wf, k, half),
            img[p0:p1, r : r + 8, c0 : c0 + W],
            start=(k == 0),
            stop=(k == 8),
        )


def _dedupe_ldweights(nc):
    """Post-compile pass: delete InstLdweights that would reload the exact
    weights already resident in that PE row group.

    The bass compile pipeline splits every matmul into InstLdweights +
    InstMatmult(ldweights=False) but never dedupes, so k-outer loops that
    reuse a stationary weight block across several matmuls still reload it
    each time.  Post-scheduling the instruction order is frozen, so tracking
    per-row-group load state over the linear stream is exact.  Deletion is
    conservative: only bf16 loads (f32/f32r non-self-loading matmuls are
    broken in walrus) with no semaphore waits/updates attached.  HW-level
    safety: the PE reorder window never pulls an LDW ahead of an in-flight
    matmul on a conflicting row group, so queue order == weight-state order.
    """
    n_del = 0
    for fn in nc.m.functions:
        for blk in fn.blocks:
            insts = list(blk.instructions)
            loaded = {}
            drop = set()
            for ins in insts:
                tn = type(ins).__name__
                if tn == "InstLdweights":
                    w = list(ins.ins)[0]
                    if str(w.dtype) != "dt.bfloat16":
                        loaded.clear()
                        continue
                    rg = (ins.tile_position or (0, 0))[0]
                    sig = (str(w), str(ins.tile_position), str(ins.tile_size))
                    si = ins.sync_info
                    clean = si is None or (not si.on_wait and not si.on_update)
                    if loaded.get(rg) == sig and clean:
                        drop.add(id(ins))
                        n_del += 1
                    else:
                        loaded[rg] = sig
                elif tn in ("InstMatmult", "InstMatmultMx"):
                    pass  # compiled matmuls are non-self-loading
                elif tn == "InstEventSemaphore":
                    pass
                elif getattr(ins, "engine", None) == mybir.EngineType.PE or tn in (
                    "InstCall",
                    "InstUnconditionalBranch",
                    "InstISA",
                ):
                    loaded.clear()
            if drop:
                blk.instructions = [i for i in insts if id(i) not in drop]
    return n_del


def build_program():
    key = (
        MM_DTYPE, SCHEME, OUT_DTYPE, LOOP_ITERS, DH0_UPPER_PACK, PROBE, LDW_OPT,
        DEDUP_LDW,
    )
    if key in _PROG_CACHE:
        return _PROG_CACHE[key]
    img_rows = HP if SCHEME == "dual" else HS_ROWS
    nc = bacc.Bacc("TRN2", target_bir_lowering=False, debug=False)
    xin_d = nc.dram_tensor(
        "xin", [NBR, 2, CIN, img_rows, WP], _mm_dt(), kind="ExternalInput"
    ).ap()
    w1x_d = nc.dram_tensor("w1x", [CIN, 9], F32, kind="ExternalInput").ap()
    w1y_d = nc.dram_tensor("w1y", [CIN, 9], F32, kind="ExternalInput").ap()
    w2xT_d = nc.dram_tensor("w2xT", [CIN, COUT], F32, kind="ExternalInput").ap()
    w2yT_d = nc.dram_tensor("w2yT", [CIN, COUT], F32, kind="ExternalInput").ap()
    out_d = nc.dram_tensor(
        "out", [NBR, COUT, H, W], _out_dt(), kind="ExternalOutput"
    ).ap()
    with tile.TileContext(nc) as tc:
        _emit(nc, tc, xin_d, w1x_d, w1y_d, w2xT_d, w2yT_d, out_d)
    nc.compile()
    if DEDUP_LDW:
        _dedupe_ldweights(nc)
    _PROG_CACHE[key] = nc
    return nc


def marshal_inputs(x, w1x, w1y, w2x, w2y):
    """Host-side data marshaling: shard over batch, zero-pad, build the
    per-partition-half copies for the selected scheme."""
    ndt = _np_in_dt()
    x = np.asarray(x, dtype=np.float32)
    xp = np.zeros((B, NBR, CIN, HP, WP), np.float32)
    xp[:, :, :, 1 : H + 1, 1 : W + 1] = x
    if SCHEME == "dual":
        xin = np.zeros((B, NBR, 2, CIN, HP, WP), ndt)
        xin[:, :, 0] = xp.astype(ndt)
        xin[:, :, 1, :, 0 : HP - 2, :] = xp[:, :, :, 2:HP, :].astype(ndt)
    else:
        xin = np.empty((B, NBR, 2, CIN, HS_ROWS, WP), ndt)
        xin[:, :, 0] = xp[:, :, :, 0:HS_ROWS, :].astype(ndt)
        xin[:, :, 1] = xp[:, :, :, HP - HS_ROWS : HP, :].astype(ndt)
    w2xT = np.ascontiguousarray(np.asarray(w2x, np.float32).T)
    w2yT = np.ascontiguousarray(np.asarray(w2y, np.float32).T)
    w1x = np.ascontiguousarray(w1x, np.float32)
    w1y = np.ascontiguousarray(w1y, np.float32)
    return [
        {
            "xin": np.ascontiguousarray(xin[i]),
            "w1x": w1x,
            "w1y": w1y,
            "w2xT": w2xT,
            "w2yT": w2yT,
        }
        for i in range(B)
    ]


def kernel(x, w1x, w1y, w2x, w2y):
    global LAST_EXEC_TIME_NS, LAST_RESULTS
    nc = build_program()
    in_maps = marshal_inputs(x, w1x, w1y, w2x, w2y)
    res = bass_utils.run_bass_kernel_spmd(
        nc, in_maps, list(range(N_CORES)), trace=TRACE
    )
    LAST_EXEC_TIME_NS = res.exec_time_ns
    LAST_RESULTS = res
    out = np.stack(
        [np.asarray(res.results[i]["out"], np.float32) for i in range(N_CORES)], axis=0
    )
    return out



# revision 31
# speedup vs baseline: 2.7057x; 1.0863x over previous
"""Trainium2 Bass kernel for nn_ComplexConv2Deffangle4Dxy.

Reference math (per batch b, branch br):
    out[br] = pointwise(w2, depthwise3x3(w1, img[br]))   with zero padding P=1
      br=0 (rot): weights (w1n, w2n) where wn = (wx+wy)^2 / sum((wx+wy)^2)
      br=1 (abs): log-domain: exp(branch(log(img + EPS), w1n, w2n))
      br=2 (x):   weights (w1x, w2x)
      br=3 (y):   weights (w1y, w2y)

Kernel strategy (per NeuronCore, data-parallel over batch B=8 -> 8 cores):
  Fuse depthwise+pointwise into a single 3x3 conv whose weights are the
  outer product  Wf[o, c, k] = w2[o, c] * w1[c, k], computed as
  PSUM-accumulated matmuls over the 9 kernel offsets with
  lhsT = fused weights (K=Cin, M=Cout=128) and rhs = shifted image views.
  Images are zero-padded on the host (pure marshaling) so every shifted
  view is a plain strided AP with no boundary special cases; for the abs
  branch Ln(x*1+EPS) maps the zero padding to log(EPS), exactly matching
  the reference's pad-then-log order.  Weight normalization for the
  rot/abs branches is computed on device (sum via ones-matmul, reciprocal
  on DVE, scale folded into the fused conv weights).

  Scheme "hpair" (default, fastest measured): partitions 0..63 hold padded
  image rows 0..34, partitions 64..127 hold rows 31..65.  Per 8-row output
  band pair, the lower-half tile (rows 8t..) and upper-half tile (rows
  32+8t..) are emitted matmul-by-matmul interleaved so consecutive K=64
  matmuls land on disjoint PE row groups and overlap (measured ~175 ns/MM
  vs ~508 ns/MM for a serial same-row-group chain; the 2.4 GHz stream
  floor is 213 ns).  bf16 operands (max rel err vs fp32 reference 4.4e-3,
  gate 2e-2), bf16 outputs (halves out-DMA bytes; host upcasts), branches
  ordered (x, abs, y, rot) so the ACT-heavy Exp evacuation overlaps x/y
  matmuls, and each band's output DMA is issued right after its PSUM
  evacuation so the kernel tail is one band, not one branch.

  Measured on-HW per-iteration (device-side For_i repeat, two-point
  wall-clock): baseline f32r/hsplit 133 us -> hpair/bf16 48.6 us.

  Findings from HW microbenchmarks (micro.py), kept for future tuning:
    - every nc.tensor.matmul is compiled into InstLdweights + non-self-
      loading InstMatmult; _dedupe_ldweights() can safely drop reloads of
      the already-resident weights (verified bit-correct on HW) but weight
      reuse does NOT speed up the stream -- the ~177 ns/MM pair floor is
      LDW-independent (pair_r36_b8 181 vs pair_b8 177 ns/MM).
    - serial same-row-group chains cost ~508-584 ns/MM regardless of LDW
      count; alternating row halves is the single biggest lever.
    - K=128 matmuls run at ~281 ns/MM with 8-bank cycling (140 ns per
      contracted offset) but the odd dh=0 offset row cannot be paired
      (mixing K=64 upper-half matmuls with K=128 in one PSUM accumulation
      group crashes TRN2), so a dual-style scheme does not win.
    - k-outer orderings (8 live PSUM accumulators) measured slower than
      band-pair ping-pong despite 4x weight reuse (houter 58.5 us).

  Older schemes kept for A/B: "dual", "hsplit" (see git of prior session),
  "houter", "hldw".
"""

import sys

for _p in ("/opt/trn_rl_repo",):
    if _p not in sys.path:
        sys.path.insert(0, _p)

import ml_dtypes
import numpy as np

import concourse.bacc as bacc
import concourse.mybir as mybir
import concourse.tile as tile
from concourse import bass_utils

F32 = mybir.dt.float32
F32R = mybir.dt.float32r
BF16 = mybir.dt.bfloat16

EPS = 1e-6
N_CORES = 8
B, NBR, CIN, COUT, H, W = 8, 4, 64, 128, 64, 64
HP, WP = H + 2, W + 2          # host-padded image
HS_ROWS = 35                   # hsplit: padded rows per partition half

# matmul input dtype: "f32r" | "f32" | "bf16"
MM_DTYPE = "bf16"
SCHEME = "hpair"               # "dual" | "hsplit" | "hpair" | "houter" | "hldw"
# output dtype on device: "f32" | "bf16" (bf16 halves the out-DMA traffic;
# host upcasts -- adds <=0.4% rounding, gate is 2e-2)
OUT_DTYPE = "bf16"
# Packing (0,+1) onto the upper PE row group (K=64 at base_partition 64)
# mixed with K=128 matmuls in the same PSUM accumulation group crashes at
# runtime on TRN2 hardware -- keep disabled.
DH0_UPPER_PACK = False
DEDUP_LDW = True               # post-compile: drop InstLdweights that reload
                               # the row-group's already-loaded weights
LOOP_ITERS = None              # benchmarking: device-side repeat count
PROBE = ""                     # "" | "no_out" (skip evac+out-DMA) | "no_mm"
TRACE = False
LAST_EXEC_TIME_NS = None
LAST_RESULTS = None

_PROG_CACHE = {}

# walrus's --enable-ldw-opt crashes at runtime on TRN2 (tested); the flag is
# kept only so older bench configs keying on it still build.
LDW_OPT = False

BRANCHES = (  # (branch index, weight set, log-domain?, evac engine)
    # abs (Exp evac, ACT-heavy) goes second so its activations overlap the
    # x/y matmul streams and the kernel tail ends on a cheap copy-evac branch
    (2, "x", False, "v"),
    (1, "n", True, "a"),
    (3, "y", False, "a"),
    (0, "n", False, "v"),
)


def _mm_dt():
    return {"f32r": F32R, "f32": F32, "bf16": BF16}[MM_DTYPE]


def _np_in_dt():
    return ml_dtypes.bfloat16 if MM_DTYPE == "bf16" else np.float32


def _out_dt():
    return BF16 if OUT_DTYPE == "bf16" else F32


def _emit(nc, tc, xin_d, w1x_d, w1y_d, w2xT_d, w2yT_d, out_d):
    import contextlib

    mdt = _mm_dt()
    img_rows = HP if SCHEME == "dual" else HS_ROWS
    # houter keeps 8 PSUM accumulators live for a whole branch -> needs all 8
    # banks in one pool; weight-prep reductions then share psp (they finish
    # before the main loop starts).
    psp_bufs = 8 if SCHEME in ("houter", "hldw") else 6
    obp_bufs = 2 if SCHEME in ("hpair", "houter", "hldw") else 6
    with contextlib.ExitStack() as _stack:
        wp = _stack.enter_context(tc.tile_pool(name="wp", bufs=1))
        imgp = _stack.enter_context(tc.tile_pool(name="imgp", bufs=2))
        psp = _stack.enter_context(tc.tile_pool(name="psp", bufs=psp_bufs, space="PSUM"))
        obp = _stack.enter_context(tc.tile_pool(name="obp", bufs=obp_bufs))
        if SCHEME in ("houter", "hldw"):
            psr, red_tag = psp, "ps"
        else:
            psr = _stack.enter_context(tc.tile_pool(name="psr", bufs=2, space="PSUM"))
            red_tag = "red"
        # ---- weight prep -------------------------------------------------
        # All weight/source tiles replicated into both partition halves so
        # per-half fused tiles can be built with partition-local DVE ops.
        w1x_s = wp.tile([2 * CIN, 9], F32, tag="w1x")
        w1y_s = wp.tile([2 * CIN, 9], F32, tag="w1y")
        w2xT_s = wp.tile([2 * CIN, COUT], F32, tag="w2xT")
        w2yT_s = wp.tile([2 * CIN, COUT], F32, tag="w2yT")
        for t, d in (
            (w1x_s, w1x_d),
            (w1y_s, w1y_d),
            (w2xT_s, w2xT_d),
            (w2yT_s, w2yT_d),
        ):
            nc.sync.dma_start(out=t[0:CIN], in_=d)
            nc.sync.dma_start(out=t[CIN : 2 * CIN], in_=d)

        ones_k = wp.tile([CIN, 1], F32, tag="ones_k")
        nc.vector.memset(ones_k[:, :], 1.0)
        ones_m = wp.tile([1, 2 * CIN], F32, tag="ones_m")
        nc.vector.memset(ones_m[:, :], 1.0)
        eps_b = wp.tile([2 * CIN, 1], F32, tag="eps_b")
        nc.vector.memset(eps_b[:, :], float(EPS))
        zero_b = wp.tile([COUT, 1], F32, tag="zero_b")
        nc.vector.memset(zero_b[:, :], 0.0)

        # u1 = (w1x + w1y)^2, u2T = ((w2x + w2y)^2)^T  (both partition halves)
        u1 = wp.tile([2 * CIN, 9], F32, tag="u1")
        nc.vector.tensor_add(u1[:, :], w1x_s[:, :], w1y_s[:, :])
        nc.vector.tensor_mul(u1[:, :], u1[:, :], u1[:, :])
        u2T = wp.tile([2 * CIN, COUT], F32, tag="u2T")
        nc.vector.tensor_add(u2T[:, :], w2xT_s[:, :], w2yT_s[:, :])
        nc.vector.tensor_mul(u2T[:, :], u2T[:, :], u2T[:, :])

        # S1 = sum(u1), S2 = sum(u2) via ones-matmul + free-dim reduce
        s1v = psr.tile([1, 9], F32, tag=red_tag)
        nc.tensor.matmul(s1v[:, :], ones_k[:, :], u1[0:CIN, :], start=True, stop=True)
        s2v = psr.tile([1, COUT], F32, tag=red_tag)
        nc.tensor.matmul(s2v[:, :], ones_k[:, :], u2T[0:CIN, :], start=True, stop=True)
        s1 = wp.tile([1, 1], F32, tag="s1")
        nc.vector.tensor_reduce(
            s1[:, :], s1v[:, :], axis=mybir.AxisListType.X, op=mybir.AluOpType.add
        )
        s2 = wp.tile([1, 1], F32, tag="s2")
        nc.vector.tensor_reduce(
            s2[:, :], s2v[:, :], axis=mybir.AxisListType.X, op=mybir.AluOpType.add
        )
        inv = wp.tile([1, 1], F32, tag="inv")
        nc.vector.tensor_mul(inv[:, :], s1[:, :], s2[:, :])
        nc.vector.reciprocal(inv[:, :], inv[:, :])
        # broadcast 1/(S1*S2) to all 128 partitions
        invb_ps = psr.tile([2 * CIN, 1], F32, tag=red_tag)
        nc.tensor.matmul(invb_ps[:, :], ones_m[:, :], inv[:, :], start=True, stop=True)
        invb = wp.tile([2 * CIN, 1], F32, tag="invb")
        nc.vector.tensor_copy(invb[:, :], invb_ps[:, :])
        # u2T_n = u2T / (S1*S2): both normalizations in one fold
        u2Tn = wp.tile([2 * CIN, COUT], F32, tag="u2Tn")
        nc.vector.tensor_scalar(
            u2Tn[:, :], u2T[:, :], invb[:, 0:1], None, mybir.AluOpType.mult
        )

        # fused weight tiles
        #  hsplit: 9 column blocks, block k = w2T*w1[:,k], same both halves
        #  dual:   6 column blocks with per-half k (see _mm_dual):
        #          slot:   0     1     2     3     4     5
        #          lower:  k0    k1    k2    k3    k4    k5
        #          upper:  k6    k7    k8    k5    -     -
        if SCHEME == "dual":
            half_ks = ((0, 1, 2, 3, 4, 5), (6, 7, 8, 5))
            n_blocks = 6
        else:
            half_ks = (tuple(range(9)), tuple(range(9)))
            n_blocks = 9
        wf_tiles = {}
        for s, base, w1s in (("x", w2xT_s, w1x_s), ("y", w2yT_s, w1y_s), ("n", u2Tn, u1)):
            wf = wp.tile([2 * CIN, n_blocks * COUT], mdt, tag=f"wf{s}")
            for half in (0, 1):
                p0, p1 = half * CIN, (half + 1) * CIN
                for slot, k in enumerate(half_ks[half]):
                    nc.vector.tensor_scalar(
                        wf[p0:p1, slot * COUT : (slot + 1) * COUT],
                        base[p0:p1, :],
                        w1s[p0:p1, k : k + 1],
                        None,
                        mybir.AluOpType.mult,
                    )
            wf_tiles[s] = wf

        # ---- main compute ------------------------------------------------
        def hpair_branch(b, s, needs_log):
            """hpair: per 8-row band, interleave the lower-half tile (out rows
            8*tpl..) with the upper-half tile (out rows 32+8*tpl..) MM-by-MM so
            consecutive matmuls land on disjoint PE row groups and overlap.
            Full branch output staged in SBUF, one DMA per branch."""
            wf = wf_tiles[s]
            img = imgp.tile([2 * CIN, img_rows, WP], mdt, tag="img")
            nc.sync.dma_start(out=img[:, :, :], in_=xin_d[b])
            if needs_log:
                nc.scalar.activation(
                    img[:, :, :],
                    img[:, :, :],
                    mybir.ActivationFunctionType.Ln,
                    bias=eps_b[:, 0:1],
                )
            ot = obp.tile([COUT, H, W], _out_dt(), tag="ot")
            for tpl in range(4):
                psL = psp.tile([COUT, 8, W], F32, tag="ps")
                psU = psp.tile([COUT, 8, W], F32, tag="ps")
                if PROBE != "no_mm":
                    for k in range(9):
                        dh, dw = k // 3 - 1, k % 3 - 1
                        c0 = 1 + dw
                        rL = 8 * tpl + 1 + dh
                        rU = rL + 1
                        nc.tensor.matmul(
                            psL[:, :, :],
                            _wfk(wf, k, 0),
                            img[0:CIN, rL : rL + 8, c0 : c0 + W],
                            start=(k == 0),
                            stop=(k == 8),
                        )
                        nc.tensor.matmul(
                            psU[:, :, :],
                            _wfk(wf, k, 1),
                            img[CIN : 2 * CIN, rU : rU + 8, c0 : c0 + W],
                            start=(k == 0),
                            stop=(k == 8),
                        )
                if PROBE == "no_out":
                    continue
                for half, ps in ((0, psL), (1, psU)):
                    h0 = 8 * tpl + 32 * half
                    dst = ot[:, h0 : h0 + 8, :]
                    if needs_log:
                        nc.scalar.activation(
                            dst, ps[:, :, :], mybir.ActivationFunctionType.Exp,
                            bias=zero_b[:, 0:1],
                        )
                    elif half == 0:
                        nc.vector.tensor_copy(dst, ps[:, :, :])
                    else:
                        nc.scalar.activation(
                            dst, ps[:, :, :], mybir.ActivationFunctionType.Copy
                        )
                    # chunked out-DMA: ship each band as soon as it is evac'd
                    # so the kernel tail is one band, not one branch
                    nc.sync.dma_start(
                        out=out_d[b, :, h0 : h0 + 8, :], in_=dst
                    )

        def houter_branch(b, s, needs_log):
            """k-outer: all 8 PSUM banks accumulate one branch; per k-offset
            the weight block stays stationary across the 4 row-bands of its
            half, with lower/upper halves interleaved MM-by-MM."""
            wf = wf_tiles[s]
            img = imgp.tile([2 * CIN, img_rows, WP], mdt, tag="img")
            nc.sync.dma_start(out=img[:, :, :], in_=xin_d[b])
            if needs_log:
                nc.scalar.activation(
                    img[:, :, :],
                    img[:, :, :],
                    mybir.ActivationFunctionType.Ln,
                    bias=eps_b[:, 0:1],
                )
            ot = obp.tile([COUT, H, W], _out_dt(), tag="ot")
            pss = [
                psp.tile([COUT, 8, W], F32, tag="ps", name=f"ps{i}")
                for i in range(8)
            ]

            def evac(tpl, half):
                ps = pss[2 * tpl + half]
                h0 = 8 * tpl + 32 * half
                dst = ot[:, h0 : h0 + 8, :]
                if needs_log:
                    nc.scalar.activation(
                        dst, ps[:, :, :], mybir.ActivationFunctionType.Exp,
                        bias=zero_b[:, 0:1],
                    )
                elif half == 0:
                    nc.vector.tensor_copy(dst, ps[:, :, :])
                else:
                    nc.scalar.activation(
                        dst, ps[:, :, :], mybir.ActivationFunctionType.Copy
                    )

            def mm(k, tpl, half):
                dh, dw = k // 3 - 1, k % 3 - 1
                r0 = 8 * tpl + 1 + dh + half
                p0 = half * CIN
                nc.tensor.matmul(
                    pss[2 * tpl + half][:, :, :],
                    _wfk(wf, k, half),
                    img[p0 : p0 + CIN, r0 : r0 + 8, 1 + dw : 1 + dw + W],
                    start=(k == 0),
                    stop=(k == 8),
                )

            if PROBE != "no_mm":
                for k in range(8):
                    for tpl in range(4):
                        mm(k, tpl, 0)
                        mm(k, tpl, 1)
                # last k-round: evac each band right after its closing matmul
                # so PSUM drains overlap the round instead of bursting at the
                # branch boundary
                for tpl in range(4):
                    for half in (0, 1):
                        mm(8, tpl, half)
                        if PROBE != "no_out":
                            evac(tpl, half)
            elif PROBE != "no_out":
                for tpl in range(4):
                    for half in (0, 1):
                        evac(tpl, half)
            if PROBE == "no_out":
                return
            nc.sync.dma_start(out=out_d[b], in_=ot[:, :, :])

        # weight-register WAR fences for hldw: per half, the matmuls still
        # consuming the currently-loaded weights; the next ldweights for that
        # half must not be scheduled above them.
        _ldw_fence = {0: [], 1: []}

        def hldw_branch(b, s, needs_log):
            """k-outer with explicit LDWEIGHTS: each (k, half) weight block is
            loaded once and consumed by 4 non-self-loading matmuls (the 4
            row-bands of its half), halves interleaved MM-by-MM."""
            from concourse.tile_rust import add_dep_helper

            wf = wf_tiles[s]
            img = imgp.tile([2 * CIN, img_rows, WP], mdt, tag="img")
            nc.sync.dma_start(out=img[:, :, :], in_=xin_d[b])
            if needs_log:
                nc.scalar.activation(
                    img[:, :, :],
                    img[:, :, :],
                    mybir.ActivationFunctionType.Ln,
                    bias=eps_b[:, 0:1],
                )
            ot = obp.tile([COUT, H, W], _out_dt(), tag="ot")
            pss = [
                psp.tile([COUT, 8, W], F32, tag="ps", name=f"ps{i}")
                for i in range(8)
            ]
            if PROBE != "no_mm":
                for k in range(9):
                    dh, dw = k // 3 - 1, k % 3 - 1
                    c0 = 1 + dw
                    ldw = {}
                    for half in (0, 1):
                        ldw[half] = nc.tensor.ldweights(
                            _wfk(wf, k, half), tile_position=(64 * half, 0)
                        )
                        for m in _ldw_fence[half]:
                            add_dep_helper(
                                ldw[half].ins, m, reason="weight WAR fence"
                            )
                        _ldw_fence[half] = []
                    for tpl in range(4):
                        rL = 8 * tpl + 1 + dh
                        for half, r0 in ((0, rL), (1, rL + 1)):
                            p0 = half * CIN
                            mm = nc.tensor.matmul(
                                pss[2 * tpl + half][:, :, :],
                                _wfk(wf, k, half),
                                img[p0 : p0 + CIN, r0 : r0 + 8, c0 : c0 + W],
                                start=(k == 0),
                                stop=(k == 8),
                            )
                            mm.ins.ldweights = False
                            add_dep_helper(
                                mm.ins, ldw[half].ins, reason="mm after its ldw"
                            )
                            _ldw_fence[half].append(mm.ins)
            if PROBE == "no_out":
                return
            for tpl in range(4):
                for half in (0, 1):
                    ps = pss[2 * tpl + half]
                    h0 = 8 * tpl + 32 * half
                    dst = ot[:, h0 : h0 + 8, :]
                    if needs_log:
                        nc.scalar.activation(
                            dst, ps[:, :, :], mybir.ActivationFunctionType.Exp,
                            bias=zero_b[:, 0:1],
                        )
                    elif half == 0:
                        nc.vector.tensor_copy(dst, ps[:, :, :])
                    else:
                        nc.scalar.activation(
                            dst, ps[:, :, :], mybir.ActivationFunctionType.Copy
                        )
            nc.sync.dma_start(out=out_d[b], in_=ot[:, :, :])

        def main_body():
            for b, s, needs_log, evac in BRANCHES:
                if SCHEME == "hpair":
                    hpair_branch(b, s, needs_log)
                    continue
                if SCHEME == "houter":
                    houter_branch(b, s, needs_log)
                    continue
                if SCHEME == "hldw":
                    hldw_branch(b, s, needs_log)
                    continue
                wf = wf_tiles[s]
                img = imgp.tile([2 * CIN, img_rows, WP], mdt, tag="img")
                nc.sync.dma_start(out=img[:, :, :], in_=xin_d[b])
                if needs_log:
                    nc.scalar.activation(
                        img[:, :, :],
                        img[:, :, :],
                        mybir.ActivationFunctionType.Ln,
                        bias=eps_b[:, 0:1],
                    )
                for tp in range(8):
                    ps = psp.tile([COUT, 8, W], F32, tag="ps")
                    if PROBE != "no_mm":
                        if SCHEME == "dual":
                            _mm_dual(nc, ps, wf, img, tp)
                        else:
                            _mm_hsplit(nc, ps, wf, img, tp)
                    if PROBE == "no_out":
                        continue
                    ot = obp.tile([COUT, 8, W], _out_dt(), tag="ot")
                    h0 = 8 * tp
                    if needs_log:
                        nc.scalar.activation(
                            ot[:, :, :],
                            ps[:, :, :],
                            mybir.ActivationFunctionType.Exp,
                            bias=zero_b[:, 0:1],
                        )
                    elif evac == "v":
                        nc.vector.tensor_copy(ot[:, :, :], ps[:, :, :])
                    else:
                        nc.scalar.activation(
                            ot[:, :, :], ps[:, :, :], mybir.ActivationFunctionType.Copy
                        )
                    nc.sync.dma_start(out=out_d[b, :, h0 : h0 + 8, :], in_=ot[:, :, :])

        if LOOP_ITERS:
            with tc.For_i(0, LOOP_ITERS, 1):
                main_body()
        else:
            main_body()


def _wfk(wf, k, half):
    p0, p1 = half * CIN, (half + 1) * CIN
    return wf[p0:p1, k * COUT : (k + 1) * COUT]


def _mm_dual(nc, ps, wf, img, tp):
    """out rows 8*tp..8*tp+7 from dual-copy image: partitions 0..63 hold the
    padded image A (rows 0..65), partitions 64..127 hold B with B[r]=A[r+2].

    6 matmuls per tile: 3x K=128 (offset pairs (-1,dw)+(+1,dw)), then the
    dh=0 row as K=64 matmuls -- (0,-1) on the lower row group packed with
    (0,+1) on the upper row group (concurrent), plus (0,0) on the lower."""
    h0 = 8 * tp
    n_mm = 6
    idx = [0]

    def step(lhsT, rhs):
        nc.tensor.matmul(
            ps[:, :, :], lhsT, rhs, start=(idx[0] == 0), stop=(idx[0] == n_mm - 1)
        )
        idx[0] += 1

    for dw in (-1, 0, 1):  # slots 0..2: K=128, lower k=dw+1, upper k=7+dw
        step(
            wf[:, (dw + 1) * COUT : (dw + 2) * COUT],
            img[:, h0 : h0 + 8, 1 + dw : 1 + dw + W],
        )
    # (0,-1) lower (slot3 low) ++ (0,+1) upper (slot3 high, B[h0-1]=A[h0+1])
    step(wf[0:CIN, 3 * COUT : 4 * COUT], img[0:CIN, h0 + 1 : h0 + 9, 0:W])
    if DH0_UPPER_PACK and tp > 0:
        step(
            wf[CIN : 2 * CIN, 3 * COUT : 4 * COUT],
            img[CIN : 2 * CIN, h0 - 1 : h0 + 7, 2 : 2 + W],
        )
    else:  # B row -1 unavailable (tp=0) or packing disabled: lower, slot 5
        step(wf[0:CIN, 5 * COUT : 6 * COUT], img[0:CIN, h0 + 1 : h0 + 9, 2 : 2 + W])
    # (0,0) lower (slot4 low)
    step(wf[0:CIN, 4 * COUT : 5 * COUT], img[0:CIN, h0 + 1 : h0 + 9, 1 : 1 + W])


def _mm_hsplit(nc, ps, wf, img, tp):
    """hsplit scheme: tile tp covers out rows 8*tp..+7; lower tiles (tp<4)
    read partitions 0..63, upper tiles read 64..127."""
    half = 0 if tp < 4 else 1
    p0, p1 = half * CIN, (half + 1) * CIN
    tpl = tp % 4
    for k in range(9):
        dh, dw = k // 3 - 1, k % 3 - 1
        r = 8 * tpl + 1 + dh + half  # lower: pad row - 0; upper: pad row - 31
        c0 = 1 + dw
        nc.tensor.matmul(
            ps[:, :, :],
            _wfk(wf, k, half),
            img[p0:p1, r : r + 8, c0 : c0 + W],
            start=(k == 0),
            stop=(k == 8),
        )


def _dedupe_ldweights(nc):
    """Post-compile pass: delete InstLdweights that would reload the exact
    weights already resident in that PE row group.

    The bass compile pipeline splits every matmul into InstLdweights +
    InstMatmult(ldweights=False) but never dedupes, so k-outer loops that
    reuse a stationary weight block across several matmuls still reload it
    each time.  Post-scheduling the instruction order is frozen, so tracking
    per-row-group load state over the linear stream is exact.  Deletion is
    conservative: only bf16 loads (f32/f32r non-self-loading matmuls are
    broken in walrus) with no semaphore waits/updates attached.  HW-level
    safety: the PE reorder window never pulls an LDW ahead of an in-flight
    matmul on a conflicting row group, so queue order == weight-state order.
    """
    n_del = 0
    for fn in nc.m.functions:
        for blk in fn.blocks:
            insts = list(blk.instructions)
            loaded = {}
            drop = set()
            for ins in insts:
                tn = type(ins).__name__
                if tn == "InstLdweights":
                    w = list(ins.ins)[0]
                    if str(w.dtype) != "dt.bfloat16":
                        loaded.clear()
                        continue
                    rg = (ins.tile_position or (0, 0))[0]
                    sig = (str(w), str(ins.tile_position), str(ins.tile_size))
                    si = ins.sync_info
                    clean = si is None or (not si.on_wait and not si.on_update)
                    if loaded.get(rg) == sig and clean:
                        drop.add(id(ins))
                        n_del += 1
                    else:
                        loaded[rg] = sig
                elif tn in ("InstMatmult", "InstMatmultMx"):
                    pass  # compiled matmuls are non-self-loading
                elif tn == "InstEventSemaphore":
                    pass
                elif getattr(ins, "engine", None) == mybir.EngineType.PE or tn in (
                    "InstCall",
                    "InstUnconditionalBranch",
                    "InstISA",
                ):
                    loaded.clear()
            if drop:
                blk.instructions = [i for i in insts if id(i) not in drop]
    return n_del


def build_program():
    key = (
        MM_DTYPE, SCHEME, OUT_DTYPE, LOOP_ITERS, DH0_UPPER_PACK, PROBE, LDW_OPT,
        DEDUP_LDW,
    )
    if key in _PROG_CACHE:
        return _PROG_CACHE[key]
    img_rows = HP if SCHEME == "dual" else HS_ROWS
    nc = bacc.Bacc("TRN2", target_bir_lowering=False, debug=False)
    xin_d = nc.dram_tensor(
        "xin", [NBR, 2 * CIN, img_rows, WP], _mm_dt(), kind="ExternalInput"
    ).ap()
    w1x_d = nc.dram_tensor("w1x", [CIN, 9], F32, kind="ExternalInput").ap()
    w1y_d = nc.dram_tensor("w1y", [CIN, 9], F32, kind="ExternalInput").ap()
    w2xT_d = nc.dram_tensor("w2xT", [CIN, COUT], F32, kind="ExternalInput").ap()
    w2yT_d = nc.dram_tensor("w2yT", [CIN, COUT], F32, kind="ExternalInput").ap()
    out_d = nc.dram_tensor(
        "out", [NBR, COUT, H, W], _out_dt(), kind="ExternalOutput"
    ).ap()
    with tile.TileContext(nc) as tc:
        _emit(nc, tc, xin_d, w1x_d, w1y_d, w2xT_d, w2yT_d, out_d)
    nc.compile()
    if DEDUP_LDW:
        _dedupe_ldweights(nc)
    _PROG_CACHE[key] = nc
    return nc


def marshal_inputs(x, w1x, w1y, w2x, w2y):
    """Host-side data marshaling: shard over batch, zero-pad, build the
    per-partition-half copies for the selected scheme."""
    ndt = _np_in_dt()
    x = np.asarray(x, dtype=np.float32)
    xp = np.zeros((B, NBR, CIN, HP, WP), np.float32)
    xp[:, :, :, 1 : H + 1, 1 : W + 1] = x
    if SCHEME == "dual":
        xin = np.zeros((B, NBR, 2, CIN, HP, WP), ndt)
        xin[:, :, 0] = xp.astype(ndt)
        xin[:, :, 1, :, 0 : HP - 2, :] = xp[:, :, :, 2:HP, :].astype(ndt)
    else:
        xin = np.empty((B, NBR, 2, CIN, HS_ROWS, WP), ndt)
        xin[:, :, 0] = xp[:, :, :, 0:HS_ROWS, :].astype(ndt)
        xin[:, :, 1] = xp[:, :, :, HP - HS_ROWS : HP, :].astype(ndt)
    w2xT = np.ascontiguousarray(np.asarray(w2x, np.float32).T)
    w2yT = np.ascontiguousarray(np.asarray(w2y, np.float32).T)
    w1x = np.ascontiguousarray(w1x, np.float32)
    w1y = np.ascontiguousarray(w1y, np.float32)
    return [
        {
            "xin": np.ascontiguousarray(xin[i]).reshape(NBR, 2 * CIN, -1, WP),
            "w1x": w1x,
            "w1y": w1y,
            "w2xT": w2xT,
            "w2yT": w2yT,
        }
        for i in range(B)
    ]


def kernel(x, w1x, w1y, w2x, w2y):
    global LAST_EXEC_TIME_NS, LAST_RESULTS
    nc = build_program()
    in_maps = marshal_inputs(x, w1x, w1y, w2x, w2y)
    res = bass_utils.run_bass_kernel_spmd(
        nc, in_maps, list(range(N_CORES)), trace=TRACE
    )
    LAST_EXEC_TIME_NS = res.exec_time_ns
    LAST_RESULTS = res
    out = np.stack(
        [np.asarray(res.results[i]["out"], np.float32) for i in range(N_CORES)], axis=0
    )
    return out

